# revision 29
# baseline (speedup 1.0000x reference)
"""Bass/Trainium2 kernel for nn_Attention_53128745452063.

Math (reference):
    dec_t = decoder_hidden @ W_dec.T                  [B,1,H]
    enc_t = encoder_outputs @ W_enc.T                 [B,S,H]
    comb  = tanh(dec_t + enc_t)                       [B,S,H]
    align = comb @ w_align.T                          [B,S,1]
    scores = softmax(align.flatten()).reshape(B,S)    (global softmax!)
    context = bmm(scores, encoder_outputs)            [B,1,H]
    returns (context, scores)

Strategy: data-parallel over batch across 8 cores (8 batches/core).
Each core computes, per batch b: raw logits a[s], local max m_b,
u = exp(a - m_b), and unnormalized context C_b = sum_s u_s * enc_s.
Host combines with the standard online-softmax rescale:
    M = max m_b;  Z = sum_b exp(m_b - M) * sum(u_b)
    scores_b = u_b * exp(m_b - M) / Z;  context_b = C_b * exp(m_b-M) / Z
which reproduces the global flattened softmax exactly.

Device kernel per core:
  The PE contracts over the partition dim, so the W_enc matmul needs enc
  with h on partitions while the context matmul needs s on partitions.
  Instead of transposing 16 MiB on-chip (PE transposes + PSUM evacuation
  dominated the first version), the HOST sends enc in BOTH layouts as
  fp16 — 8 MiB transposed + 8 MiB natural = the same DMA bytes as one
  fp32 copy. fp16 keeps ~5e-4 relative accuracy (PSUM accumulates fp32;
  tanh/align/softmax all stay fp32).

  Per 512-row tile: matmul encT slice against pre-transposed W_enc
  (fp16 x fp16 -> fp32 PSUM), tanh with fused per-partition bias
  (host-precomputed dec_t.T) -> SBUF fp32 combT, then per-128-chunk
  matmul against w_align.T -> aT logits [128, 16] per batch in PSUM
  (s on partitions). Per batch: DVE free-dim max + GPSIMD
  partition_all_reduce -> broadcast max, ACT exp(a - max) PSUM->SBUF,
  DVE cast of u to fp16, then 16 accumulating context matmuls against
  the resident natural-layout fp16 enc chunks.

  Emission is software-pipelined over 16 tile-pairs: each pair's align
  matmuls are emitted one pair late so the PE queue always has
  independent yT work in front of tanh-dependent instructions, and the
  DMA queue sends each batch's transposed layout one batch ahead of its
  natural layout. Built with bacc.Bacc (its generate_event_semaphores
  pass splits multi-wait instructions, which walrus can't encode on
  matmuls).
"""

import numpy as np

B, S, H = 64, 2048, 256
NCORES = 8
BPC = B // NCORES          # batches per core = 8
ROWS = BPC * S             # rows per core = 16384
CPB = S // 128             # 128-row chunks per batch = 16
NCHUNK = ROWS // 128       # chunks per core = 128

_cache = {}


def _build():
    import concourse.mybir as mybir
    from concourse import bacc, bass_isa
    from concourse.bass import _add_dep_helper
    from concourse.tile import TileContext

    fp32 = mybir.dt.float32
    fp16 = mybir.dt.float16
    AX = mybir.AxisListType
    AF = mybir.ActivationFunctionType

    nc = bacc.Bacc(None)
    encT_in = nc.declare_dram_parameter("encT", [H, ROWS], fp16, isOutput=False)
    encN_in = nc.declare_dram_parameter("encN", [ROWS, H], fp16, isOutput=False)
    wencT_in = nc.declare_dram_parameter("wencT", [H, H], fp16, isOutput=False)
    dectT_in = nc.declare_dram_parameter("dectT", [H, BPC], fp32, isOutput=False)
    walT_in = nc.declare_dram_parameter("walT", [H, 1], fp16, isOutput=False)
    u_out = nc.declare_dram_parameter("u_out", [128, NCHUNK], fp32, isOutput=True)
    cm_out = nc.declare_dram_parameter("cm_out", [1, BPC * H + BPC], fp32, isOutput=True)

    with TileContext(nc) as tc:
        with (
            tc.tile_pool(name="res", bufs=1) as res,
            tc.tile_pool(name="sb_combT", bufs=8) as p_combT_sb,
            tc.tile_pool(name="sm", bufs=4) as sm,
            tc.tile_pool(name="ps_yT", bufs=2, space="PSUM") as p_yT_ps,
            tc.tile_pool(name="ps_aT", bufs=1, space="PSUM") as p_aT_ps,
            tc.tile_pool(name="ps_ctx", bufs=2, space="PSUM") as p_ctx_ps,
        ):
            # ---- DMA queue order: small weights, then batch-0 encT (gates
            # the first matmul), then per-batch with encT one batch AHEAD
            # of encN (encN_b is only needed by ctx(b), one pair later) ----
            encT_re = encT_in[:].rearrange("(k p) s -> p k s", p=128)  # [128,2,ROWS]
            encN_re = encN_in[:].rearrange("(c p) h -> p c h", p=128)  # [128,128,256]

            wencT_sb = res.tile([128, 2, H], fp16)      # [h%128, h//128, h_out]
            dma_wencT = nc.sync.dma_start(
                out=wencT_sb, in_=wencT_in[:].rearrange("(k p) o -> p k o", p=128)
            )
            dectT_sb = res.tile([128, 2, BPC], fp32)    # [h_out%128, h_out//128, b]
            nc.sync.dma_start(
                out=dectT_sb, in_=dectT_in[:].rearrange("(k p) b -> p k b", p=128)
            )
            walT_sb = res.tile([128, 2, 1], fp16)
            dma_walT = nc.sync.dma_start(
                out=walT_sb, in_=walT_in[:].rearrange("(k p) o -> p k o", p=128)
            )

            uT_sb = res.tile([128, NCHUNK], fp32)       # exp(a - m_b), col=chunk
            uT16 = res.tile([128, NCHUNK], fp16)        # fp16 copy for ctx lhsT
            cm_sb = res.tile([1, BPC * H + BPC], fp32)  # [C_b cols | m_b cols]

            encT_b, encN_b = [], []
            dma_encT, dma_encN = [], []
            tT0 = res.tile([128, 2, S], fp16, name="encT_0", tag="encT_0")
            dma_encT0a = nc.sync.dma_start(
                out=tT0[:, :, 0 : S // 2], in_=encT_re[:, :, 0 : S // 2]
            )
            dma_encT.append(dma_encT0a)
            encT_b.append(tT0)
            dma_encT0b = nc.sync.dma_start(
                out=tT0[:, :, S // 2 : S], in_=encT_re[:, :, S // 2 : S]
            )
            # interleave: encT_{b+1} before encN_b
            for b in range(BPC):
                if b + 1 < BPC:
                    tT = res.tile(
                        [128, 2, S], fp16, name=f"encT_{b+1}", tag=f"encT_{b+1}"
                    )
                    dma_encT.append(
                        nc.sync.dma_start(
                            out=tT, in_=encT_re[:, :, (b + 1) * S : (b + 2) * S]
                        )
                    )
                    encT_b.append(tT)
                tN = res.tile([128, CPB, H], fp16, name=f"encN_{b}", tag=f"encN_{b}")
                dma_encN.append(
                    nc.sync.dma_start(
                        out=tN, in_=encN_re[:, b * CPB : (b + 1) * CPB, :]
                    )
                )
                encN_b.append(tN)

            # ---- main loop: software-pipelined emission ----
            # PE queue per batch: yT0 yT1 aT0 ctx(b-1) yT2 aT1 yT3 aT2 aT3
            # so no PE instruction waits at the head of the queue while
            # independent work is available behind it.
            def emit_yT(b, t, yT_ps, hook, m2s=(0, 1)):
                for m2 in m2s:
                    for k2 in range(2):
                        mm = nc.tensor.matmul(
                            yT_ps[m2][:, t % 2, :],
                            wencT_sb[:, k2, m2 * 128 : (m2 + 1) * 128],
                            encT_b[b][:, k2, t * 512 : (t + 1) * 512],
                            start=(k2 == 0),
                            stop=(k2 == 1),
                        )
                        if hook and m2 == 1:
                            if k2 == 0:
                                _add_dep_helper(mm.ins, dma_encN[b].ins, sync=True)
                            elif b + 1 < BPC:
                                _add_dep_helper(
                                    mm.ins, dma_encT[b + 1].ins, sync=True
                                )

            def emit_tanh(b, t, yT_ps, combT_sb):
                for m2 in range(2):
                    nc.scalar.activation(
                        combT_sb[m2],
                        yT_ps[m2],
                        AF.Tanh,
                        bias=dectT_sb[:, m2, b : b + 1],
                        scale=1.0,
                    )

            def emit_aT(b, t, aT_ps, combT_sb):
                for jj in range(4):
                    cloc = t * 4 + jj
                    for m2 in range(2):
                        nc.tensor.matmul(
                            aT_ps[:, cloc : cloc + 1],
                            combT_sb[m2][:, t % 2, jj * 128 : (jj + 1) * 128],
                            walT_sb[:, m2, :],
                            start=(m2 == 0),
                            stop=(m2 == 1),
                        )

            def emit_stats(b, aT_ps):
                colmax = sm.tile([128, 1], fp32, name=f"colmax_{b}", tag="colmax")
                nc.vector.reduce_max(out=colmax, in_=aT_ps, axis=AX.X)
                maxbc = sm.tile([128, 1], fp32, name=f"maxbc_{b}", tag="maxbc")
                nc.gpsimd.partition_all_reduce(
                    maxbc, colmax, channels=128, reduce_op=bass_isa.ReduceOp.max
                )
                nc.vector.tensor_copy(cm_sb[:, BPC * H + b : BPC * H + b + 1], maxbc[0:1, :])
                negpre = sm.tile([128, 1], fp32, name=f"negpre_{b}", tag="negpre")
                nc.vector.tensor_scalar_mul(negpre, maxbc, -1.0)
                nc.scalar.activation(
                    uT_sb[:, b * CPB : (b + 1) * CPB],
                    aT_ps,
                    AF.Exp,
                    bias=negpre,
                    scale=1.0,
                )
                nc.vector.tensor_copy(
                    uT16[:, b * CPB : (b + 1) * CPB],
                    uT_sb[:, b * CPB : (b + 1) * CPB],
                )
                nc.sync.dma_start(
                    out=u_out[:, b * CPB : (b + 1) * CPB],
                    in_=uT_sb[:, b * CPB : (b + 1) * CPB],
                )

            def emit_ctx(b):
                ctx_ps = p_ctx_ps.tile([1, H], fp32, name=f"ctx_{b}", tag="ctx")
                for j in range(CPB):
                    nc.tensor.matmul(
                        ctx_ps,
                        uT16[:, b * CPB + j : b * CPB + j + 1],
                        encN_b[b][:, j, :],
                        start=(j == 0),
                        stop=(j == CPB - 1),
                    )
                nc.vector.tensor_copy(cm_sb[:, b * H : (b + 1) * H], ctx_ps)

            # Flat pipeline over 16 pairs; each pair's aT matmuls are
            # emitted ONE PAIR LATE so the PE queue always has independent
            # yT work in front of the tanh-dependent aT matmuls.
            aT_tiles = {}
            pend = None  # (b, pr, aT_ps, combT)
            for i in range(2 * BPC):
                b, pr = divmod(i, 2)
                if pr == 0:
                    aT_tiles[b] = p_aT_ps.tile(
                        [128, CPB], fp32, name=f"aT_{b}", tag=f"aT{b % 2}"
                    )
                yT = [
                    p_yT_ps.tile([128, 2, 512], fp32, name=f"yT_ps_{b}_{pr}_{k}", tag="yT_ps")
                    for k in range(2)
                ]
                combT = [
                    p_combT_sb.tile([128, 2, 512], fp16, name=f"combT_sb_{b}_{pr}_{k}", tag="combT_sb")
                    for k in range(2)
                ]
                t0, t1 = 2 * pr, 2 * pr + 1
                emit_yT(b, t0, yT, hook=False, m2s=(0,))
                emit_yT(b, t1, yT, hook=(pr == 0), m2s=(0,))
                emit_yT(b, t0, yT, hook=False, m2s=(1,))
                emit_yT(b, t1, yT, hook=False, m2s=(1,))
                emit_tanh(b, pr, yT, combT)
                if pend is not None:
                    pb, ppr, pa, pc = pend
                    emit_aT(pb, 2 * ppr, pa, pc)
                    emit_aT(pb, 2 * ppr + 1, pa, pc)
                    if ppr == 1 and pb > 0:
                        emit_ctx(pb - 1)
                if pr == 0 and b > 0:
                    # after the lagged aT(b-1, p1) matmuls above
                    emit_stats(b - 1, aT_tiles[b - 1])
                pend = (b, pr, aT_tiles[b], combT)
            pb, ppr, pa, pc = pend
            emit_aT(pb, 2 * ppr, pa, pc)
            emit_aT(pb, 2 * ppr + 1, pa, pc)
            emit_ctx(BPC - 2)
            emit_stats(BPC - 1, aT_tiles[BPC - 1])
            emit_ctx(BPC - 1)

            # ---- outputs ----
            nc.sync.dma_start(out=cm_out[:], in_=cm_sb)

    nc.compile()
    return nc


def _get_nc():
    if "nc" not in _cache:
        _cache["nc"] = _build()
    return _cache["nc"]


def _prepare_in_maps(inputs):
    enc = np.ascontiguousarray(np.asarray(inputs["encoder_outputs"], dtype=np.float32))
    dec = np.asarray(inputs["decoder_hidden"], dtype=np.float32)
    W_dec = np.asarray(inputs["W_dec"], dtype=np.float32)
    W_enc = np.asarray(inputs["W_enc"], dtype=np.float32)
    w_al = np.asarray(inputs["w_align"], dtype=np.float32)

    dec_t = dec @ W_dec.T                      # [B, H]
    wencT = np.ascontiguousarray(W_enc.T.astype(np.float16))   # [H, H]
    walT = np.ascontiguousarray(w_al.reshape(1, H).T.astype(np.float16))
    enc16 = enc.astype(np.float16)

    in_maps = []
    for i in range(NCORES):
        sl = slice(i * BPC, (i + 1) * BPC)
        shard = enc16[sl].reshape(ROWS, H)
        in_maps.append(
            {
                "encT": np.ascontiguousarray(shard.T),
                "encN": shard,
                "wencT": wencT,
                "dectT": np.ascontiguousarray(dec_t[sl].T),
                "walT": walT,
            }
        )
    return enc, in_maps


def _combine(enc, results):
    # Gather per-core outputs and apply the global softmax rescale on host.
    u = np.empty((B, S), dtype=np.float32)
    C = np.empty((B, H), dtype=np.float32)
    m = np.empty((B,), dtype=np.float32)
    for i in range(NCORES):
        r = results[i]
        uT = r["u_out"]                              # [128, 128]
        u_shard = uT.reshape(128, BPC, CPB).transpose(1, 2, 0).reshape(BPC, S)
        u[i * BPC : (i + 1) * BPC] = u_shard
        cm = r["cm_out"][0]
        C[i * BPC : (i + 1) * BPC] = cm[: BPC * H].reshape(BPC, H)
        m[i * BPC : (i + 1) * BPC] = cm[BPC * H :]

    Mg = m.max()
    coef = np.exp(m - Mg)                            # [B]
    Z = float((coef * u.sum(axis=1)).sum())
    scores = u * (coef / Z)[:, None]
    context = (C * (coef / Z)[:, None])[:, None, :]  # [B, 1, H]
    return context.astype(np.float32), scores.astype(np.float32)


def run(inputs, trace=False):
    from concourse.bass_utils import run_bass_kernel_spmd

    nc = _get_nc()
    enc, in_maps = _prepare_in_maps(inputs)
    res = run_bass_kernel_spmd(nc, in_maps, list(range(NCORES)), trace=trace)
    return _combine(enc, res.results), res


def kernel(**inputs):
    out, _ = run(inputs)
    return out


# revision 30
# speedup vs baseline: 1.0055x; 1.0055x over previous
"""Bass/Trainium2 kernel for nn_Attention_53128745452063.

Math (reference):
    dec_t = decoder_hidden @ W_dec.T                  [B,1,H]
    enc_t = encoder_outputs @ W_enc.T                 [B,S,H]
    comb  = tanh(dec_t + enc_t)                       [B,S,H]
    align = comb @ w_align.T                          [B,S,1]
    scores = softmax(align.flatten()).reshape(B,S)    (global softmax!)
    context = bmm(scores, encoder_outputs)            [B,1,H]
    returns (context, scores)

Strategy: data-parallel over batch across 8 cores (8 batches/core).
Each core computes, per batch b: raw logits a[s], local max m_b,
u = exp(a - m_b), and unnormalized context C_b = sum_s u_s * enc_s.
Host combines with the standard online-softmax rescale:
    M = max m_b;  Z = sum_b exp(m_b - M) * sum(u_b)
    scores_b = u_b * exp(m_b - M) / Z;  context_b = C_b * exp(m_b-M) / Z
which reproduces the global flattened softmax exactly.

Device kernel per core:
  The PE contracts over the partition dim, so the W_enc matmul needs enc
  with h on partitions while the context matmul needs s on partitions.
  Instead of transposing 16 MiB on-chip (PE transposes + PSUM evacuation
  dominated the first version), the HOST sends enc in BOTH layouts as
  fp16 — 8 MiB transposed + 8 MiB natural = the same DMA bytes as one
  fp32 copy. fp16 keeps ~5e-4 relative accuracy (PSUM accumulates fp32;
  tanh/align/softmax all stay fp32).

  Per 512-row tile: matmul encT slice against pre-transposed W_enc
  (fp16 x fp16 -> fp32 PSUM), tanh with fused per-partition bias
  (host-precomputed dec_t.T) -> SBUF fp32 combT, then per-128-chunk
  matmul against w_align.T -> aT logits [128, 16] per batch in PSUM
  (s on partitions). Per batch: DVE free-dim max + GPSIMD
  partition_all_reduce -> broadcast max, ACT exp(a - max) PSUM->SBUF,
  DVE cast of u to fp16, then 16 accumulating context matmuls against
  the resident natural-layout fp16 enc chunks.

  Emission is software-pipelined over 16 tile-pairs: each pair's align
  matmuls are emitted one pair late so the PE queue always has
  independent yT work in front of tanh-dependent instructions, and the
  DMA queue sends each batch's transposed layout one batch ahead of its
  natural layout. Built with bacc.Bacc (its generate_event_semaphores
  pass splits multi-wait instructions, which walrus can't encode on
  matmuls).
"""

import numpy as np

B, S, H = 64, 2048, 256
NCORES = 8
BPC = B // NCORES          # batches per core = 8
ROWS = BPC * S             # rows per core = 16384
CPB = S // 128             # 128-row chunks per batch = 16
NCHUNK = ROWS // 128       # chunks per core = 128

_cache = {}


def _build():
    import concourse.mybir as mybir
    from concourse import bacc, bass_isa
    from concourse.bass import _add_dep_helper
    from concourse.tile import TileContext

    fp32 = mybir.dt.float32
    fp16 = mybir.dt.float16
    AX = mybir.AxisListType
    AF = mybir.ActivationFunctionType

    nc = bacc.Bacc(None)
    encT_in = nc.declare_dram_parameter("encT", [H, ROWS], fp16, isOutput=False)
    encN_in = nc.declare_dram_parameter("encN", [ROWS, H], fp16, isOutput=False)
    wencT_in = nc.declare_dram_parameter("wencT", [H, H], fp16, isOutput=False)
    dectT_in = nc.declare_dram_parameter("dectT", [H, BPC], fp32, isOutput=False)
    walT_in = nc.declare_dram_parameter("walT", [H, 1], fp16, isOutput=False)
    u_out = nc.declare_dram_parameter("u_out", [128, NCHUNK], fp32, isOutput=True)
    cm_out = nc.declare_dram_parameter("cm_out", [1, BPC * H + BPC], fp32, isOutput=True)

    with TileContext(nc) as tc:
        with (
            tc.tile_pool(name="res", bufs=1) as res,
            tc.tile_pool(name="sb_combT", bufs=8) as p_combT_sb,
            tc.tile_pool(name="sm", bufs=4) as sm,
            tc.tile_pool(name="ps_yT", bufs=2, space="PSUM") as p_yT_ps,
            tc.tile_pool(name="ps_aT", bufs=1, space="PSUM") as p_aT_ps,
            tc.tile_pool(name="ps_ctx", bufs=2, space="PSUM") as p_ctx_ps,
        ):
            # ---- DMA queue order: small weights, then batch-0 encT (gates
            # the first matmul), then per-batch with encT one batch AHEAD
            # of encN (encN_b is only needed by ctx(b), one pair later) ----
            encT_re = encT_in[:].rearrange("(k p) s -> p k s", p=128)  # [128,2,ROWS]
            encN_re = encN_in[:].rearrange("(c p) h -> p c h", p=128)  # [128,128,256]

            wencT_sb = res.tile([128, 2, H], fp16)      # [h%128, h//128, h_out]
            dma_wencT = nc.sync.dma_start(
                out=wencT_sb, in_=wencT_in[:].rearrange("(k p) o -> p k o", p=128)
            )
            dectT_sb = res.tile([128, 2, BPC], fp32)    # [h_out%128, h_out//128, b]
            nc.sync.dma_start(
                out=dectT_sb, in_=dectT_in[:].rearrange("(k p) b -> p k b", p=128)
            )
            walT_sb = res.tile([128, 2, 1], fp16)
            dma_walT = nc.sync.dma_start(
                out=walT_sb, in_=walT_in[:].rearrange("(k p) o -> p k o", p=128)
            )

            uT_sb = res.tile([128, NCHUNK], fp32)       # exp(a - m_b), col=chunk
            uT16 = res.tile([128, NCHUNK], fp16)        # fp16 copy for ctx lhsT
            cm_sb = res.tile([1, BPC * H + BPC], fp32)  # [C_b cols | m_b cols]

            encT_b, encN_b = [], []
            dma_encT, dma_encN = [], []
            tT0 = res.tile([128, 2, S], fp16, name="encT_0", tag="encT_0")
            dma_encT0a = nc.sync.dma_start(
                out=tT0[:, :, 0 : S // 4], in_=encT_re[:, :, 0 : S // 4]
            )
            dma_encT.append(dma_encT0a)
            encT_b.append(tT0)
            dma_encT0b = nc.sync.dma_start(
                out=tT0[:, :, S // 4 : S], in_=encT_re[:, :, S // 4 : S]
            )
            # interleave: encT_{b+1} before encN_b
            for b in range(BPC):
                if b + 1 < BPC:
                    tT = res.tile(
                        [128, 2, S], fp16, name=f"encT_{b+1}", tag=f"encT_{b+1}"
                    )
                    dma_encT.append(
                        nc.sync.dma_start(
                            out=tT, in_=encT_re[:, :, (b + 1) * S : (b + 2) * S]
                        )
                    )
                    encT_b.append(tT)
                tN = res.tile([128, CPB, H], fp16, name=f"encN_{b}", tag=f"encN_{b}")
                dma_encN.append(
                    nc.sync.dma_start(
                        out=tN, in_=encN_re[:, b * CPB : (b + 1) * CPB, :]
                    )
                )
                encN_b.append(tN)

            # ---- main loop: software-pipelined emission ----
            # PE queue per batch: yT0 yT1 aT0 ctx(b-1) yT2 aT1 yT3 aT2 aT3
            # so no PE instruction waits at the head of the queue while
            # independent work is available behind it.
            def emit_yT(b, t, yT_ps, hook, m2s=(0, 1)):
                for m2 in m2s:
                    for k2 in range(2):
                        mm = nc.tensor.matmul(
                            yT_ps[m2][:, t % 2, :],
                            wencT_sb[:, k2, m2 * 128 : (m2 + 1) * 128],
                            encT_b[b][:, k2, t * 512 : (t + 1) * 512],
                            start=(k2 == 0),
                            stop=(k2 == 1),
                        )
                        if hook and m2 == 1:
                            if k2 == 0:
                                _add_dep_helper(mm.ins, dma_encN[b].ins, sync=True)
                            elif b + 1 < BPC:
                                _add_dep_helper(
                                    mm.ins, dma_encT[b + 1].ins, sync=True
                                )

            def emit_tanh(b, t, yT_ps, combT_sb):
                for m2 in range(2):
                    nc.scalar.activation(
                        combT_sb[m2],
                        yT_ps[m2],
                        AF.Tanh,
                        bias=dectT_sb[:, m2, b : b + 1],
                        scale=1.0,
                    )

            def emit_aT(b, t, aT_ps, combT_sb):
                for jj in range(4):
                    cloc = t * 4 + jj
                    for m2 in range(2):
                        nc.tensor.matmul(
                            aT_ps[:, cloc : cloc + 1],
                            combT_sb[m2][:, t % 2, jj * 128 : (jj + 1) * 128],
                            walT_sb[:, m2, :],
                            start=(m2 == 0),
                            stop=(m2 == 1),
                        )

            def emit_stats(b, aT_ps):
                colmax = sm.tile([128, 1], fp32, name=f"colmax_{b}", tag="colmax")
                nc.vector.reduce_max(out=colmax, in_=aT_ps, axis=AX.X)
                maxbc = sm.tile([128, 1], fp32, name=f"maxbc_{b}", tag="maxbc")
                nc.gpsimd.partition_all_reduce(
                    maxbc, colmax, channels=128, reduce_op=bass_isa.ReduceOp.max
                )
                nc.vector.tensor_copy(cm_sb[:, BPC * H + b : BPC * H + b + 1], maxbc[0:1, :])
                negpre = sm.tile([128, 1], fp32, name=f"negpre_{b}", tag="negpre")
                nc.vector.tensor_scalar_mul(negpre, maxbc, -1.0)
                nc.scalar.activation(
                    uT_sb[:, b * CPB : (b + 1) * CPB],
                    aT_ps,
                    AF.Exp,
                    bias=negpre,
                    scale=1.0,
                )
                nc.vector.tensor_copy(
                    uT16[:, b * CPB : (b + 1) * CPB],
                    uT_sb[:, b * CPB : (b + 1) * CPB],
                )
                nc.sync.dma_start(
                    out=u_out[:, b * CPB : (b + 1) * CPB],
                    in_=uT_sb[:, b * CPB : (b + 1) * CPB],
                )

            def emit_ctx(b):
                ctx_ps = p_ctx_ps.tile([1, H], fp32, name=f"ctx_{b}", tag="ctx")
                for j in range(CPB):
                    nc.tensor.matmul(
                        ctx_ps,
                        uT16[:, b * CPB + j : b * CPB + j + 1],
                        encN_b[b][:, j, :],
                        start=(j == 0),
                        stop=(j == CPB - 1),
                    )
                nc.vector.tensor_copy(cm_sb[:, b * H : (b + 1) * H], ctx_ps)

            # Flat pipeline over 16 pairs; each pair's aT matmuls are
            # emitted ONE PAIR LATE so the PE queue always has independent
            # yT work in front of the tanh-dependent aT matmuls.
            aT_tiles = {}
            pend = None  # (b, pr, aT_ps, combT)
            for i in range(2 * BPC):
                b, pr = divmod(i, 2)
                if pr == 0:
                    aT_tiles[b] = p_aT_ps.tile(
                        [128, CPB], fp32, name=f"aT_{b}", tag=f"aT{b % 2}"
                    )
                yT = [
                    p_yT_ps.tile([128, 2, 512], fp32, name=f"yT_ps_{b}_{pr}_{k}", tag="yT_ps")
                    for k in range(2)
                ]
                combT = [
                    p_combT_sb.tile([128, 2, 512], fp16, name=f"combT_sb_{b}_{pr}_{k}", tag="combT_sb")
                    for k in range(2)
                ]
                t0, t1 = 2 * pr, 2 * pr + 1
                emit_yT(b, t0, yT, hook=False, m2s=(0,))
                emit_yT(b, t1, yT, hook=(pr == 0), m2s=(0,))
                emit_yT(b, t0, yT, hook=False, m2s=(1,))
                emit_yT(b, t1, yT, hook=False, m2s=(1,))
                emit_tanh(b, pr, yT, combT)
                if pend is not None:
                    pb, ppr, pa, pc = pend
                    emit_aT(pb, 2 * ppr, pa, pc)
                    emit_aT(pb, 2 * ppr + 1, pa, pc)
                    if ppr == 1 and pb > 0:
                        emit_ctx(pb - 1)
                if pr == 0 and b > 0:
                    # after the lagged aT(b-1, p1) matmuls above
                    emit_stats(b - 1, aT_tiles[b - 1])
                pend = (b, pr, aT_tiles[b], combT)
            pb, ppr, pa, pc = pend
            emit_aT(pb, 2 * ppr, pa, pc)
            emit_aT(pb, 2 * ppr + 1, pa, pc)
            emit_ctx(BPC - 2)
            emit_stats(BPC - 1, aT_tiles[BPC - 1])
            emit_ctx(BPC - 1)

            # ---- outputs ----
            nc.sync.dma_start(out=cm_out[:], in_=cm_sb)

    nc.compile()
    return nc


def _get_nc():
    if "nc" not in _cache:
        _cache["nc"] = _build()
    return _cache["nc"]


def _prepare_in_maps(inputs):
    enc = np.ascontiguousarray(np.asarray(inputs["encoder_outputs"], dtype=np.float32))
    dec = np.asarray(inputs["decoder_hidden"], dtype=np.float32)
    W_dec = np.asarray(inputs["W_dec"], dtype=np.float32)
    W_enc = np.asarray(inputs["W_enc"], dtype=np.float32)
    w_al = np.asarray(inputs["w_align"], dtype=np.float32)

    dec_t = dec @ W_dec.T                      # [B, H]
    wencT = np.ascontiguousarray(W_enc.T.astype(np.float16))   # [H, H]
    walT = np.ascontiguousarray(w_al.reshape(1, H).T.astype(np.float16))
    enc16 = enc.astype(np.float16)

    in_maps = []
    for i in range(NCORES):
        sl = slice(i * BPC, (i + 1) * BPC)
        shard = enc16[sl].reshape(ROWS, H)
        in_maps.append(
            {
                "encT": np.ascontiguousarray(shard.T),
                "encN": shard,
                "wencT": wencT,
                "dectT": np.ascontiguousarray(dec_t[sl].T),
                "walT": walT,
            }
        )
    return enc, in_maps


def _combine(enc, results):
    # Gather per-core outputs and apply the global softmax rescale on host.
    u = np.empty((B, S), dtype=np.float32)
    C = np.empty((B, H), dtype=np.float32)
    m = np.empty((B,), dtype=np.float32)
    for i in range(NCORES):
        r = results[i]
        uT = r["u_out"]                              # [128, 128]
        u_shard = uT.reshape(128, BPC, CPB).transpose(1, 2, 0).reshape(BPC, S)
        u[i * BPC : (i + 1) * BPC] = u_shard
        cm = r["cm_out"][0]
        C[i * BPC : (i + 1) * BPC] = cm[: BPC * H].reshape(BPC, H)
        m[i * BPC : (i + 1) * BPC] = cm[BPC * H :]

    Mg = m.max()
    coef = np.exp(m - Mg)                            # [B]
    Z = float((coef * u.sum(axis=1)).sum())
    scores = u * (coef / Z)[:, None]
    context = (C * (coef / Z)[:, None])[:, None, :]  # [B, 1, H]
    return context.astype(np.float32), scores.astype(np.float32)


def run(inputs, trace=False):
    from concourse.bass_utils import run_bass_kernel_spmd

    nc = _get_nc()
    enc, in_maps = _prepare_in_maps(inputs)
    res = run_bass_kernel_spmd(nc, in_maps, list(range(NCORES)), trace=trace)
    return _combine(enc, res.results), res


def kernel(**inputs):
    out, _ = run(inputs)
    return out
